# revision 22
# baseline (speedup 1.0000x reference)
"""Trainium2 Bass kernel for CausalCrossAttention (B=8, T=769, C=1024, H=16).

Sharding: data-parallel over batch B=8 across the 8 NeuronCores (one batch
element per core, SPMD).

v1 (bf16 rewrite of the fp32r baseline):
  - All matmul operands bf16 (host-cast); PSUM accumulates fp32. Halves HBM
    traffic (the baseline lost ~80us waiting on fp32 weight DMAs) and lets a
    single matmul stream the full 770-col T range (bf16 moving max = 1024).
  - Q/K projections in [c_out, t] layout with partial rotary via a host
    permutation (even/odd pair split per head) + partition-block-swap DMAs +
    3 DVE ops (bf16 = 2x DVE rate).
  - S^T attention row-tiled: head pair (2j, 2j+1) computed CONCURRENTLY by two
    K=64 matmuls in disjoint PE row groups (tile_position auto from
    base_partition 0 / 64) -> S^T costs N cycles per head PAIR, and the
    baseline's qz sibling-zeroing copies disappear.
  - Per (j, nk): both heads' S^T go to one 4-bank psum tile [128, 2, 1024]
    (bank-disjoint halves), ONE 3D-AP exp covers both heads (halves ACT
    instruction overhead; ACT is the attention-phase bottleneck).
  - PV with M=66 per head (64 v-dims + ones column for the softmax
    denominator + 1 pad col for evenness), accumulated in [128, 2, 1024].
  - Softmax division: denominator rows DMA'd from PSUM with partition
    broadcast, one approx reciprocal per head pair, two DVE mults.
  - Output projection in [c_out, t] layout (host transposes back).
"""

import os

import numpy as np

B, T, C = 8, 769, 1024
H, HD, L = 16, 64, 32
COND = 256
NCI = 8
NCO = 8
TP = 770
HW = HD + 2  # per-head vaug block: 64 v dims + ones col + pad col
VW = H * HW  # 1056

_CACHE = {}

# Per-(kv-tile) q ranges + mask offset: kv tile nk covers cols
# [128nk, 128nk+128); allowed iff kv_col < 256 + q_col.
R0SUB = {0: (0, None), 1: (0, None), 2: (0, 0), 3: (128, 128),
         4: (256, 256), 5: (384, 384)}


def _build_program():
    import concourse.mybir as mybir
    import concourse.tile as tile
    from concourse import bacc

    f32 = mybir.dt.float32
    bf16 = mybir.dt.bfloat16
    Exp = mybir.ActivationFunctionType.Exp
    Ident = mybir.ActivationFunctionType.Identity

    nc = bacc.Bacc("TRN2", target_bir_lowering=False)

    xqT_d = nc.dram_tensor("xqT", [C, TP], bf16, kind="ExternalInput")
    xkvT_d = nc.dram_tensor("xkvT", [C, TP], bf16, kind="ExternalInput")
    wq_d = nc.dram_tensor("wqT", [C, C], bf16, kind="ExternalInput")
    wk_d = nc.dram_tensor("wkT", [C, C], bf16, kind="ExternalInput")
    wv_d = nc.dram_tensor("wvT", [C, C], bf16, kind="ExternalInput")
    wp_d = nc.dram_tensor("wpT", [C, C], bf16, kind="ExternalInput")
    bq_d = nc.dram_tensor("bq2", [128, NCO], f32, kind="ExternalInput")
    bk_d = nc.dram_tensor("bk2", [128, NCO], f32, kind="ExternalInput")
    bp_d = nc.dram_tensor("bp2", [128, NCO], f32, kind="ExternalInput")
    bv_d = nc.dram_tensor("bv1", [1, C], f32, kind="ExternalInput")
    cos_d = nc.dram_tensor("cosP", [128, TP], bf16, kind="ExternalInput")
    sin_d = nc.dram_tensor("sinP", [128, TP], bf16, kind="ExternalInput")
    m0_d = nc.dram_tensor("m0", [128, 128], bf16, kind="ExternalInput")
    out_d = nc.dram_tensor("outT", [C, TP], bf16, kind="ExternalOutput")

    with tile.TileContext(nc) as tc:
        with (
            tc.tile_pool(name="consts", bufs=1) as consts,
            tc.tile_pool(name="wpool", bufs=1) as wpool,
            tc.tile_pool(name="qkpool", bufs=1) as qkpool,
            tc.tile_pool(name="vpool", bufs=1) as vpool,
        ):
            cos_sb = consts.tile([128, TP], bf16, tag="cos")
            sin_sb = consts.tile([128, TP], bf16, tag="sin")
            m0_sb = consts.tile([128, 128], bf16, tag="m0")
            bq_sb = consts.tile([128, NCO], f32, tag="bq")
            bk_sb = consts.tile([128, NCO], f32, tag="bk")
            bp_sb = consts.tile([128, NCO], f32, tag="bp")
            ones16 = consts.tile([128, 16], f32, tag="ones16")
            nc.vector.memset(ones16, 1.0)
            dly = consts.tile([1, 8], bf16, tag="dly")
            zcol = consts.tile([128, 32], f32, tag="zcol")
            nc.vector.memset(zcol, 0.0)
            nc.scalar.dma_start(out=cos_sb, in_=cos_d[:, :])
            nc.scalar.dma_start(out=sin_sb, in_=sin_d[:, :])
            nc.scalar.dma_start(out=m0_sb, in_=m0_d[:, :])
            nc.scalar.dma_start(out=bq_sb, in_=bq_d[:, :])
            nc.scalar.dma_start(out=bk_sb, in_=bk_d[:, :])
            nc.scalar.dma_start(out=bp_sb, in_=bp_d[:, :])

            qT = qkpool.tile([128, NCI, TP], bf16, tag="qT")
            kT = qkpool.tile([128, NCI, TP], bf16, tag="kT")
            vaug = vpool.tile([128, 7, VW], bf16, tag="vaug")
            yT = None

            def load_w(wdram, pfx, q):
                ws = []
                for ci in range(NCI):
                    wt = wpool.tile([128, C], bf16, tag=f"{pfx}{ci}",
                                    name=f"{pfx}{ci}")
                    q.dma_start(
                        out=wt, in_=wdram[ci * 128:(ci + 1) * 128, :])
                    ws.append(wt)
                return ws

            def proj_qk(w, x, b_sb, outT, psA, shpool,
                        groups=(range(0, 4), range(4, 8))):
                for cog in groups:
                    pss = {}
                    for co in cog:
                        pss[co] = psA.tile([128, 1024], f32, tag="ps",
                                           name=f"psqk{co}")
                    for ci in range(NCI):
                        for co in cog:
                            for (lo, hi) in ((0, 512), (512, TP)):
                                nc.tensor.matmul(
                                    pss[co][:, lo:hi],
                                    w[ci][:, co * 128:(co + 1) * 128],
                                    x[:, ci, lo:hi],
                                    start=(ci == 0), stop=(ci == NCI - 1))
                    for co in cog:
                        proj_qk_tail(pss[co], co, b_sb, outT, shpool)

            def proj_qk_tail(ps, co, b_sb, outT, shpool):
                # bias add + PSUM->SBUF(bf16) on DVE, keeping the ACT engine
                # (and its sequencer) free to start attention exps early.
                nc.vector.tensor_scalar_add(
                    out=outT[:, co, :], in0=ps[:, 0:TP],
                    scalar1=b_sb[:, co:co + 1])
                # rotary: swap 16-row blocks of the (host-permuted) rotary
                # dims, then q = q*cos + swapped*sin (cos rows outside the
                # rotary dims are 1.0, sin rows are 0.0).
                sh = shpool.tile([128, TP], bf16, tag="sh", name="sh")
                nc.sync.dma_start(
                    out=sh[32:64, :], in_=outT[32:64, co, :])
                for s in (0, 64):
                    nc.sync.dma_start(
                        out=sh[s:s + 16, :], in_=outT[s + 16:s + 32, co, :])
                    nc.sync.dma_start(
                        out=sh[s + 16:s + 32, :], in_=outT[s:s + 16, co, :])
                nc.vector.tensor_mul(
                    sh[0:96, :], sh[0:96, :], sin_sb[0:96, :])
                nc.vector.tensor_mul(
                    outT[:, co, :], outT[:, co, :], cos_sb)
                nc.vector.tensor_add(
                    outT[0:96, co, :], outT[0:96, co, :], sh[0:96, :])

            def proj_v(w, x, psA, bv_sb,
                       groups=(range(0, 4), range(4, 7))):
                for g in groups:
                    pss = {}
                    for tt in g:
                        pss[tt] = psA.tile([128, 1024], f32, tag="ps",
                                           name=f"psv{tt}")
                    for ci in range(NCI):
                        for tt in g:
                            tsz = 128 if tt < 6 else 1
                            for hf in (0, 1):
                                nc.tensor.matmul(
                                    pss[tt][:tsz, hf * 512:hf * 512 + 512],
                                    x[:, ci, tt * 128:tt * 128 + tsz],
                                    w[ci][:, hf * 512:hf * 512 + 512],
                                    start=(ci == 0), stop=(ci == NCI - 1))
                    for tt in g:
                        tsz = 128 if tt < 6 else 1
                        va = vaug[:tsz, tt, :].rearrange(
                            "p (h e) -> p h e", e=HW)
                        nc.vector.tensor_add(
                            va[:, :, 0:HD],
                            pss[tt][:tsz, :].rearrange("p (h d) -> p h d", h=H),
                            bv_sb[:tsz, :].rearrange("p (h d) -> p h d", h=H))
                        nc.vector.tensor_copy(
                            va[:, :, HD:HD + 1], ones16[:tsz, :].unsqueeze(2))
                        nc.vector.tensor_copy(
                            va[:, :, HD + 1:HW], zcol[:tsz, 0:16].unsqueeze(2))

            def proj_v_tt(w, x, tt, pool, tag, bv_sb):
                tsz = 128 if tt < 6 else 1
                ps = pool.tile([128, 2, 512], f32, tag=tag,
                               name=f"psv{tt}")
                for ci in range(NCI):
                    for hf in (0, 1):
                        nc.tensor.matmul(
                            ps[:tsz, hf, :],
                            x[:, ci, tt * 128:tt * 128 + tsz],
                            w[ci][:, hf * 512:hf * 512 + 512],
                            start=(ci == 0), stop=(ci == NCI - 1))
                va = vaug[:tsz, tt, :].rearrange("p (h e) -> p h e", e=HW)
                nc.vector.tensor_add(
                    va[:, :, 0:HD],
                    ps[:tsz, :, :].rearrange("p r c -> p (r c)")
                        .rearrange("p (h d) -> p h d", h=H),
                    bv_sb[:tsz, :].rearrange("p (h d) -> p h d", h=H))
                nc.vector.tensor_copy(
                    va[:, :, HD:HD + 1], ones16[:tsz, :].unsqueeze(2))
                nc.vector.tensor_copy(
                    va[:, :, HD + 1:HW], zcol[:tsz, 0:16].unsqueeze(2))

            def st_chunk(j, nk, pts, psS, pt_pool):
                """Emit S^T + exp for one (pair, kv-block) chunk."""
                if nk < 6:
                    qlo, moff = R0SUB[nk]
                    ks = slice(nk * 128, (nk + 1) * 128)
                    st0 = psS.tile([128, 2, 512], f32, tag="stR0",
                                   name=f"stR0_{j}_{nk}")
                    pt0 = pt_pool.tile([128, 2, 512], bf16,
                                       tag=f"ptR0{nk}", bufs=2,
                                       name=f"ptR0_{j}_{nk}")
                    nc.tensor.matmul(
                        st0[:, 0, qlo:512], kT[0:64, j, ks],
                        qT[0:64, j, qlo:512], start=True, stop=True)
                    nc.tensor.matmul(
                        st0[:, 1, qlo:512], kT[64:128, j, ks],
                        qT[64:128, j, qlo:512], start=True, stop=True)
                    nc.scalar.activation(
                        out=pt0[:, :, qlo:512], in_=st0[:, :, qlo:512],
                        func=Exp, scale=0.125)
                    if moff is not None:
                        nc.gpsimd.tensor_mul(
                            pt0[:, 0, moff:moff + 128],
                            pt0[:, 0, moff:moff + 128], m0_sb)
                        nc.gpsimd.tensor_mul(
                            pt0[:, 1, moff:moff + 128],
                            pt0[:, 1, moff:moff + 128], m0_sb)
                    st1 = psS.tile([128, 2, 512], f32, tag="stR1",
                                   name=f"stR1_{j}_{nk}")
                    pt1 = pt_pool.tile([128, 2, 512], bf16,
                                       tag=f"ptR1{nk}", bufs=2,
                                       name=f"ptR1_{j}_{nk}")
                    nc.tensor.matmul(
                        st1[:, 0, 0:258], kT[0:64, j, ks],
                        qT[0:64, j, 512:TP], start=True, stop=True)
                    nc.tensor.matmul(
                        st1[:, 1, 0:258], kT[64:128, j, ks],
                        qT[64:128, j, 512:TP], start=True, stop=True)
                    nc.scalar.activation(
                        out=pt1[:, :, 0:258], in_=st1[:, :, 0:258],
                        func=Exp, scale=0.125)
                    pts[nk] = (pt0, pt1)
                else:
                    # kv row 768: q cols 512:770 only, col 512 masked
                    st1 = psS.tile([128, 2, 512], f32, tag="stR1",
                                   name=f"stR1_{j}_{nk}")
                    pt1 = pt_pool.tile([128, 2, 512], bf16,
                                       tag="ptR16", bufs=2,
                                       name=f"ptR1_{j}_{nk}")
                    nc.tensor.matmul(
                        st1[0:1, 0, 0:258], kT[0:64, j, 768:769],
                        qT[0:64, j, 512:TP], start=True, stop=True)
                    nc.tensor.matmul(
                        st1[0:1, 1, 0:258], kT[64:128, j, 768:769],
                        qT[64:128, j, 512:TP], start=True, stop=True)
                    nc.scalar.activation(
                        out=pt1[0:1, :, 0:258], in_=st1[0:1, :, 0:258],
                        func=Exp, scale=0.125)
                    nc.gpsimd.tensor_copy(
                        pt1[0:1, :, 0:1], zcol[0:1, 0:2].unsqueeze(2))
                    pts[nk] = (None, pt1)

            def pv_chunk(j, pts, ov0, ov1, nk):
                """One kv-block of both heads' PV accumulation."""
                pt0, pt1 = pts[nk]
                for h in (0, 1):
                    vs = slice((2 * j + h) * HW, (2 * j + h) * HW + HW)
                    if nk < 6:
                        qlo = R0SUB[nk][0]
                        nc.tensor.matmul(
                            ov0[0:HW, h, qlo:512], vaug[:, nk, vs],
                            pt0[:, h, qlo:512],
                            start=(nk == 0), stop=(nk == 5))
                        nc.tensor.matmul(
                            ov1[0:HW, h, 0:258], vaug[:, nk, vs],
                            pt1[:, h, 0:258],
                            start=(nk == 0), stop=False)
                    else:
                        nc.tensor.matmul(
                            ov1[0:HW, h, 0:258], vaug[0:1, 6, vs],
                            pt1[0:1, h, 0:258],
                            start=False, stop=True)

            def division(j, ov0, ov1, rdbc_pool, dnd, pending):
                # Softmax division, decoupled from PSUM residency:
                # 1) one DVE copy per ov chunk moves rows 0:65 (y + denom)
                #    to SBUF right after the PV stop -> ov banks free ~1.5us
                #    after the last PV matmul, so the next head pair's PV is
                #    not dammed by the broadcast round trip.
                # 2) denominator rows go SBUF->DRAM->broadcast, reciprocal.
                # 3) the normalization mults for pair j run one pair LATE
                #    (emitted in pair j+1), when rdbc(j) is long since ready.
                yraw0 = yraw_pool.tile([128, 2, 512], f32, tag="yraw0",
                                       bufs=2, name=f"yraw0_{j}")
                yraw1 = yraw_pool.tile([128, 2, 258], f32, tag="yraw1",
                                       bufs=2, name=f"yraw1_{j}")
                rdbcA = rdbc_pool.tile([64, TP], f32, tag="rdbcA",
                                       name=f"rdbcA{j}")
                rdbcB = rdbc_pool.tile([64, TP], f32, tag="rdbcB",
                                       name=f"rdbcB{j}")
                nc.vector.tensor_copy(
                    yraw0[0:HD + 1, :, :], ov0[0:HD + 1, :, :])
                nc.vector.tensor_copy(
                    yraw1[0:HD + 1, :, :], ov1[0:HD + 1, :, 0:258])
                nc.sync.dma_start(
                    out=dnd[j:j + 1, :, 0:512], in_=yraw0[HD:HD + 1, :, :])
                nc.sync.dma_start(
                    out=dnd[j:j + 1, :, 512:TP], in_=yraw1[HD:HD + 1, :, :])
                nc.gpsimd.dma_start(
                    out=rdbcA[:, 0:512],
                    in_=dnd[j:j + 1, 0, 0:512].broadcast_to((64, 512)))
                nc.gpsimd.dma_start(
                    out=rdbcB[:, 0:512],
                    in_=dnd[j:j + 1, 1, 0:512].broadcast_to((64, 512)))
                nc.gpsimd.dma_start(
                    out=rdbcA[:, 512:TP],
                    in_=dnd[j:j + 1, 0, 512:TP].broadcast_to((64, 258)))
                nc.gpsimd.dma_start(
                    out=rdbcB[:, 512:TP],
                    in_=dnd[j:j + 1, 1, 512:TP].broadcast_to((64, 258)))
                nc.vector.reciprocal_approx_fast(out=rdbcA, in_=rdbcA)
                nc.vector.reciprocal_approx_fast(out=rdbcB, in_=rdbcB)
                pending.append((j, yraw0, yraw1, rdbcA, rdbcB))

            def norm_muls(j, yraw0, yraw1, rdbcA, rdbcB):
                nc.vector.tensor_mul(
                    yT[0:64, j, 0:512], yraw0[0:HD, 0, :], rdbcA[:, 0:512])
                nc.vector.tensor_mul(
                    yT[64:128, j, 0:512], yraw0[0:HD, 1, :],
                    rdbcB[:, 0:512])
                nc.vector.tensor_mul(
                    yT[0:64, j, 512:TP], yraw1[0:HD, 0, :],
                    rdbcA[:, 512:TP])
                nc.vector.tensor_mul(
                    yT[64:128, j, 512:TP], yraw1[0:HD, 1, :],
                    rdbcB[:, 512:TP])

            def proj_out(w, psS, psO, opool):
                # psum tiles reuse the attention tags (same 2-bank shape):
                # stR0/stR1 free right after the last exps, ovR0/ovR1 after
                # the last division -> out-proj overlaps attention's tail.
                tags = ((psS, "stR0"), (psS, "stR1"), (psO, "ovR0"),
                        (psO, "ovR1"))
                pss = {}
                for co in range(NCO):
                    pool, tag = tags[co % 4]
                    pss[co] = pool.tile([128, 2, 512], f32, tag=tag,
                                        name=f"pso{co}")
                    for ci in range(NCI):
                        for r, (lo, hi) in enumerate(((0, 512), (512, TP))):
                            nc.tensor.matmul(
                                pss[co][:, r, 0:hi - lo],
                                w[ci][:, co * 128:(co + 1) * 128],
                                yT[:, ci, lo:hi],
                                start=(ci == 0), stop=(ci == NCI - 1))
                    ot = opool.tile([128, TP], bf16, tag="ot", name="ot")
                    nc.scalar.activation(
                        out=ot[:, 0:512], in_=pss[co][:, 0, :], func=Ident,
                        bias=bp_sb[:, co:co + 1], scale=1.0)
                    nc.scalar.activation(
                        out=ot[:, 512:TP], in_=pss[co][:, 1, 0:258],
                        func=Ident, bias=bp_sb[:, co:co + 1], scale=1.0)
                    nc.sync.dma_start(
                        out=out_d[co * 128:(co + 1) * 128, :], in_=ot)

            # ---- phase 1: projections ----
            with (
                tc.tile_pool(name="psA", bufs=4, space="PSUM") as psA,
                tc.tile_pool(name="xq", bufs=1) as xqp,
                tc.tile_pool(name="xkv", bufs=1) as xkp,
                tc.tile_pool(name="shpool", bufs=2) as shpool,
                tc.tile_pool(name="wqkp", bufs=1) as wqkp,
            ):
                bv_sb = qkpool.tile([128, C], f32, tag="bv")
                nc.gpsimd.dma_start(
                    out=bv_sb, in_=bv_d[0:1, :].broadcast_to((128, C)))
                xq = xqp.tile([128, NCI, TP], bf16, tag="xq")
                xkv = qkpool.tile([128, NCI, TP], bf16, tag="xkv")
                # HWDGE rings are FIFO per engine: SP ring carries xq+wq
                # (per-ci interleaved) then wv; ACT ring carries xkv+wk
                # (per-ci interleaved) then wp. Need-ordering keeps the
                # critical QK-proj inputs at full HBM bandwidth instead of
                # round-robin-sharing it with the late-needed wv/wp.
                # SP ring: xq+wq per-ci interleaved (+ the rotary shuffles
                # issued on dependency). ACT ring: xkv+wk per-ci, then wv,
                # then wp — ring FIFO delays wv/wp past the critical K-proj
                # inputs without blocking any compute queue.
                wq, wk = [], []
                for ci in range(NCI):
                    nc.sync.dma_start(
                        out=xq[:, ci, :],
                        in_=xqT_d[ci * 128:(ci + 1) * 128, :])
                    wt = wqkp.tile([128, C], bf16, tag=f"wq{ci}",
                                   name=f"wq{ci}")
                    nc.sync.dma_start(
                        out=wt, in_=wq_d[ci * 128:(ci + 1) * 128, :])
                    wq.append(wt)
                    nc.scalar.dma_start(
                        out=xkv[:, ci, :],
                        in_=xkvT_d[ci * 128:(ci + 1) * 128, :])
                    wt = wqkp.tile([128, C], bf16, tag=f"wk{ci}",
                                   name=f"wk{ci}")
                    nc.scalar.dma_start(
                        out=wt, in_=wk_d[ci * 128:(ci + 1) * 128, :])
                    wk.append(wt)
                proj_qk(wq, xq, bq_sb, qT, psA, shpool)
                # Sentinel copy: depends on the finished Q co=0 rotary, so
                # the wv/wp SWDGE triggers behind it on the gpsimd queue
                # issue only ~25us in — the critical xq/wq/xkv/wk stream
                # keeps full HBM bandwidth, and wv still lands before the
                # V projection needs it.
                nc.gpsimd.tensor_copy(dly, qT[0:1, 0, 0:8])
                wv = load_w(wv_d, "wv", nc.gpsimd)
                wp = load_w(wp_d, "wp", nc.gpsimd)
                proj_qk(wk, xkv, bk_sb, kT, psA, shpool)
                wv_tiles = wv
                wp_tiles = wp
                bv_keep = bv_sb

            # ---- phases 2+3 share yT ----
            with tc.tile_pool(name="ypool", bufs=1) as ypool:
                yT = ypool.tile([128, NCI, TP], bf16, tag="yT")
                # ---- phase 2: attention ----
                with (
                    tc.tile_pool(name="psS", bufs=1, space="PSUM") as psS,
                    tc.tile_pool(name="psO", bufs=1, space="PSUM") as psO,
                    tc.tile_pool(name="ptp", bufs=1) as pt_pool,
                    tc.tile_pool(name="rdbcp", bufs=2) as rdbc_pool,
                    tc.tile_pool(name="yrawp", bufs=2) as yraw_pool,
                    tc.tile_pool(name="rddp", bufs=1, space="DRAM") as dram_pool,
                ):
                    dnd = dram_pool.tile([NCI, 2, TP], f32, tag="dnd")
                    pending = []
                    ptsd = {j: {} for j in range(NCI)}
                    # Prime: V-projection chunks (PE filler, no dependence on
                    # the K rotary) run first so the PE rides through the
                    # K-rotary pipeline drain; pairs 0-1's S^T+exp interleave
                    # behind them. VP chunks reuse the st-ring psum tags.
                    for tt in (0, 1, 2):
                        proj_v_tt(wv_tiles, xkv, tt, psS,
                                  "stR0" if tt % 2 == 0 else "stR1", bv_keep)
                    for nk in (0, 1, 2, 3):
                        st_chunk(0, nk, ptsd[0], psS, pt_pool)
                    proj_v_tt(wv_tiles, xkv, 3, psS, "stR1", bv_keep)
                    for nk in (4, 5, 6):
                        st_chunk(0, nk, ptsd[0], psS, pt_pool)
                    proj_v_tt(wv_tiles, xkv, 4, psS, "stR0", bv_keep)
                    for nk in (0, 1, 2, 3):
                        st_chunk(1, nk, ptsd[1], psS, pt_pool)
                    proj_v_tt(wv_tiles, xkv, 5, psS, "stR1", bv_keep)
                    for nk in (4, 5, 6):
                        st_chunk(1, nk, ptsd[1], psS, pt_pool)
                    proj_v_tt(wv_tiles, xkv, 6, psS, "stR0", bv_keep)
                    # Steady state: PV of pair j chunk-interleaved with
                    # S^T+exp of pair j+2, so the PE never idles while ACT
                    # (the attention-phase bottleneck) drains exp chunks.
                    for j in range(NCI):
                        ov0 = psO.tile([128, 2, 512], f32, tag="ovR0",
                                       name=f"ov0_{j}")
                        ov1 = psO.tile([128, 2, 512], f32, tag="ovR1",
                                       name=f"ov1_{j}")
                        for nk in range(7):
                            pv_chunk(j, ptsd[j], ov0, ov1, nk)
                            if j + 2 < NCI:
                                st_chunk(j + 2, nk, ptsd[j + 2], psS, pt_pool)
                        division(j, ov0, ov1, rdbc_pool, dnd, pending)
                        if len(pending) > 1:
                            norm_muls(*pending.pop(0))
                    for args in pending:
                        norm_muls(*args)

                    # ---- phase 3: output projection (attention psum tags
                    # reused so it overlaps the tail of attention) ----
                    with tc.tile_pool(name="opool", bufs=3) as opool:
                        proj_out(wp_tiles, psS, psO, opool)

    nc.compile()
    return nc


def _host_prep(x_q, x_kv, rotary_pos_emb, Wq, bq, Wk, bk, Wv, bv, Wp, bp):
    f = np.float32
    x_q = np.asarray(x_q, f)
    x_kv = np.asarray(x_kv, f)
    freqs = np.asarray(rotary_pos_emb, f)

    # Even/odd pair-split permutation of the first 32 dims of each head, so
    # rotate_half becomes a 16-partition block swap on chip.
    perm = np.arange(C)
    for h in range(H):
        b0 = h * HD
        blk = np.empty(HD, np.int64)
        blk[0:16] = b0 + np.arange(0, 32, 2)
        blk[16:32] = b0 + np.arange(1, 32, 2)
        blk[32:64] = b0 + np.arange(32, 64)
        perm[b0:b0 + HD] = blk

    def wT(W, p=None):
        W = np.asarray(W, f)
        if p is not None:
            W = W[p, :]
        return np.ascontiguousarray(W.T).astype(np.float32)

    cosE = np.cos(freqs[:, 0::2]).T
    cosO = np.cos(freqs[:, 1::2]).T
    sinE = -np.sin(freqs[:, 0::2]).T
    sinO = np.sin(freqs[:, 1::2]).T
    cosP = np.ones((128, TP), f)
    sinP = np.zeros((128, TP), f)
    for s in (0, 64):
        cosP[s:s + 16, :T] = cosE
        cosP[s + 16:s + 32, :T] = cosO
        sinP[s:s + 16, :T] = sinE
        sinP[s + 16:s + 32, :T] = sinO

    p_idx = np.arange(128)[:, None]
    f_idx = np.arange(128)[None, :]
    m0 = (p_idx < f_idx).astype(f)

    import ml_dtypes
    bf = ml_dtypes.bfloat16

    bqp = np.asarray(bq, f)[perm]
    bkp = np.asarray(bk, f)[perm]
    shared = {
        "wqT": wT(Wq, perm).astype(bf),
        "wkT": wT(Wk, perm).astype(bf),
        "wvT": wT(Wv).astype(bf),
        "wpT": wT(Wp).astype(bf),
        "bq2": np.ascontiguousarray(bqp.reshape(NCO, 128).T).astype(f),
        "bk2": np.ascontiguousarray(bkp.reshape(NCO, 128).T).astype(f),
        "bp2": np.ascontiguousarray(
            np.asarray(bp, f).reshape(NCO, 128).T).astype(f),
        "bv1": np.asarray(bv, f).reshape(1, C).copy(),
        "cosP": np.ascontiguousarray(cosP).astype(bf),
        "sinP": np.ascontiguousarray(sinP).astype(bf),
        "m0": np.ascontiguousarray(m0).astype(bf),
    }

    def padT(xt):
        out = np.zeros((C, TP), np.float32)
        out[:, :T] = xt
        return out.astype(bf)

    in_maps = []
    for b in range(B):
        m = dict(shared)
        m["xqT"] = padT(x_q[b].T)
        m["xkvT"] = padT(x_kv[b].T)
        in_maps.append(m)
    return in_maps


def kernel(x_q, x_kv, rotary_pos_emb, Wq, bq, Wk, bk, Wv, bv, Wp, bp):
    from concourse.bass_utils import run_bass_kernel_spmd

    if "nc" not in _CACHE:
        _CACHE["nc"] = _build_program()
    nc = _CACHE["nc"]

    in_maps = _host_prep(x_q, x_kv, rotary_pos_emb,
                         Wq, bq, Wk, bk, Wv, bv, Wp, bp)
    trace = os.environ.get("BTK_TRACE", "0") == "1"
    res = run_bass_kernel_spmd(
        nc, in_maps, core_ids=list(range(B)), trace=trace)
    _CACHE["last_result"] = res
    out = np.stack(
        [np.asarray(r["outT"][:, :T], np.float32).T for r in res.results],
        axis=0)
    return out



# revision 23
# speedup vs baseline: 1.1304x; 1.1304x over previous
"""Trainium2 Bass kernel for CausalCrossAttention (B=8, T=769, C=1024, H=16).

Sharding: data-parallel over batch B=8 across the 8 NeuronCores (one batch
element per core, SPMD).

v1 (bf16 rewrite of the fp32r baseline):
  - All matmul operands bf16 (host-cast); PSUM accumulates fp32. Halves HBM
    traffic (the baseline lost ~80us waiting on fp32 weight DMAs) and lets a
    single matmul stream the full 770-col T range (bf16 moving max = 1024).
  - Q/K projections in [c_out, t] layout with partial rotary via a host
    permutation (even/odd pair split per head) + partition-block-swap DMAs +
    3 DVE ops (bf16 = 2x DVE rate).
  - S^T attention row-tiled: head pair (2j, 2j+1) computed CONCURRENTLY by two
    K=64 matmuls in disjoint PE row groups (tile_position auto from
    base_partition 0 / 64) -> S^T costs N cycles per head PAIR, and the
    baseline's qz sibling-zeroing copies disappear.
  - Per (j, nk): both heads' S^T go to one 4-bank psum tile [128, 2, 1024]
    (bank-disjoint halves), ONE 3D-AP exp covers both heads (halves ACT
    instruction overhead; ACT is the attention-phase bottleneck).
  - PV with M=66 per head (64 v-dims + ones column for the softmax
    denominator + 1 pad col for evenness), accumulated in [128, 2, 1024].
  - Softmax division: denominator rows DMA'd from PSUM with partition
    broadcast, one approx reciprocal per head pair, two DVE mults.
  - Output projection in [c_out, t] layout (host transposes back).
"""

import os

import numpy as np

B, T, C = 8, 769, 1024
H, HD, L = 16, 64, 32
COND = 256
NCI = 8
NCO = 8
TP = 770
HW = HD + 2  # per-head vaug block: 64 v dims + ones col + pad col
VW = H * HW  # 1056

_CACHE = {}

# Per-(kv-tile) q ranges + mask offset: kv tile nk covers cols
# [128nk, 128nk+128); allowed iff kv_col < 256 + q_col.
R0SUB = {0: (0, None), 1: (0, None), 2: (0, 0), 3: (128, 128),
         4: (256, 256), 5: (384, 384)}


def _build_program():
    import concourse.mybir as mybir
    import concourse.tile as tile
    from concourse import bacc

    f32 = mybir.dt.float32
    bf16 = mybir.dt.bfloat16
    Exp = mybir.ActivationFunctionType.Exp
    Ident = mybir.ActivationFunctionType.Identity

    nc = bacc.Bacc("TRN2", target_bir_lowering=False)

    xqT_d = nc.dram_tensor("xqT", [C, TP], bf16, kind="ExternalInput")
    xkvT_d = nc.dram_tensor("xkvT", [C, TP], bf16, kind="ExternalInput")
    wq_d = nc.dram_tensor("wqT", [C, C], bf16, kind="ExternalInput")
    wk_d = nc.dram_tensor("wkT", [C, C], bf16, kind="ExternalInput")
    wv_d = nc.dram_tensor("wvT", [C, C], bf16, kind="ExternalInput")
    wp_d = nc.dram_tensor("wpT", [C, C], bf16, kind="ExternalInput")
    bq_d = nc.dram_tensor("bq2", [128, NCO], f32, kind="ExternalInput")
    bk_d = nc.dram_tensor("bk2", [128, NCO], f32, kind="ExternalInput")
    bp_d = nc.dram_tensor("bp2", [128, NCO], f32, kind="ExternalInput")
    bv_d = nc.dram_tensor("bv1", [1, C], f32, kind="ExternalInput")
    cos_d = nc.dram_tensor("cosP", [128, TP], bf16, kind="ExternalInput")
    sin_d = nc.dram_tensor("sinP", [128, TP], bf16, kind="ExternalInput")
    m0_d = nc.dram_tensor("m0", [128, 128], bf16, kind="ExternalInput")
    out_d = nc.dram_tensor("outT", [C, TP], bf16, kind="ExternalOutput")

    with tile.TileContext(nc) as tc:
        with (
            tc.tile_pool(name="consts", bufs=1) as consts,
            tc.tile_pool(name="wpool", bufs=1) as wpool,
            tc.tile_pool(name="qkpool", bufs=1) as qkpool,
            tc.tile_pool(name="vpool", bufs=1) as vpool,
        ):
            cos_sb = consts.tile([128, TP], bf16, tag="cos")
            sin_sb = consts.tile([128, TP], bf16, tag="sin")
            m0_sb = consts.tile([128, 128], bf16, tag="m0")
            bq_sb = consts.tile([128, NCO], f32, tag="bq")
            bk_sb = consts.tile([128, NCO], f32, tag="bk")
            bp_sb = consts.tile([128, NCO], f32, tag="bp")
            ones16 = consts.tile([128, 16], f32, tag="ones16")
            nc.vector.memset(ones16, 1.0)
            dly = consts.tile([1, 8], bf16, tag="dly")
            zcol = consts.tile([128, 32], f32, tag="zcol")
            nc.vector.memset(zcol, 0.0)
            nc.scalar.dma_start(out=cos_sb, in_=cos_d[:, :])
            nc.scalar.dma_start(out=sin_sb, in_=sin_d[:, :])
            nc.scalar.dma_start(out=m0_sb, in_=m0_d[:, :])
            nc.scalar.dma_start(out=bq_sb, in_=bq_d[:, :])
            nc.scalar.dma_start(out=bk_sb, in_=bk_d[:, :])
            nc.scalar.dma_start(out=bp_sb, in_=bp_d[:, :])

            qT = qkpool.tile([128, NCI, TP], bf16, tag="qT")
            kT = qkpool.tile([128, NCI, TP], bf16, tag="kT")
            vaug = vpool.tile([128, 7, VW], bf16, tag="vaug")
            yT = None

            def load_w(wdram, pfx, q):
                ws = []
                for ci in range(NCI):
                    wt = wpool.tile([128, C], bf16, tag=f"{pfx}{ci}",
                                    name=f"{pfx}{ci}")
                    q.dma_start(
                        out=wt, in_=wdram[ci * 128:(ci + 1) * 128, :])
                    ws.append(wt)
                return ws

            def proj_qk(w, x, b_sb, outT, psA, shpool,
                        groups=(range(0, 4), range(4, 8))):
                for cog in groups:
                    pss = {}
                    for co in cog:
                        pss[co] = psA.tile([128, 1024], f32, tag="ps",
                                           name=f"psqk{co}")
                    for ci in range(NCI):
                        for co in cog:
                            for (lo, hi) in ((0, 512), (512, TP)):
                                nc.tensor.matmul(
                                    pss[co][:, lo:hi],
                                    w[ci][:, co * 128:(co + 1) * 128],
                                    x[:, ci, lo:hi],
                                    start=(ci == 0), stop=(ci == NCI - 1))
                    for co in cog:
                        proj_qk_tail(pss[co], co, b_sb, outT, shpool)

            def proj_qk_tail(ps, co, b_sb, outT, shpool):
                # bias add + PSUM->SBUF(bf16) on DVE, keeping the ACT engine
                # (and its sequencer) free to start attention exps early.
                nc.vector.tensor_scalar_add(
                    out=outT[:, co, :], in0=ps[:, 0:TP],
                    scalar1=b_sb[:, co:co + 1])
                # rotary: swap 16-row blocks of the (host-permuted) rotary
                # dims, then q = q*cos + swapped*sin (cos rows outside the
                # rotary dims are 1.0, sin rows are 0.0).
                sh = shpool.tile([128, TP], bf16, tag="sh", name="sh")
                nc.sync.dma_start(
                    out=sh[32:64, :], in_=outT[32:64, co, :])
                for s in (0, 64):
                    nc.sync.dma_start(
                        out=sh[s:s + 16, :], in_=outT[s + 16:s + 32, co, :])
                    nc.sync.dma_start(
                        out=sh[s + 16:s + 32, :], in_=outT[s:s + 16, co, :])
                nc.vector.tensor_mul(
                    sh[0:96, :], sh[0:96, :], sin_sb[0:96, :])
                nc.vector.tensor_mul(
                    outT[:, co, :], outT[:, co, :], cos_sb)
                nc.vector.tensor_add(
                    outT[0:96, co, :], outT[0:96, co, :], sh[0:96, :])

            def proj_v(w, x, psA, bv_sb,
                       groups=(range(0, 4), range(4, 7))):
                for g in groups:
                    pss = {}
                    for tt in g:
                        pss[tt] = psA.tile([128, 1024], f32, tag="ps",
                                           name=f"psv{tt}")
                    for ci in range(NCI):
                        for tt in g:
                            tsz = 128 if tt < 6 else 1
                            for hf in (0, 1):
                                nc.tensor.matmul(
                                    pss[tt][:tsz, hf * 512:hf * 512 + 512],
                                    x[:, ci, tt * 128:tt * 128 + tsz],
                                    w[ci][:, hf * 512:hf * 512 + 512],
                                    start=(ci == 0), stop=(ci == NCI - 1))
                    for tt in g:
                        tsz = 128 if tt < 6 else 1
                        va = vaug[:tsz, tt, :].rearrange(
                            "p (h e) -> p h e", e=HW)
                        nc.vector.tensor_add(
                            va[:, :, 0:HD],
                            pss[tt][:tsz, :].rearrange("p (h d) -> p h d", h=H),
                            bv_sb[:tsz, :].rearrange("p (h d) -> p h d", h=H))
                        nc.vector.tensor_copy(
                            va[:, :, HD:HD + 1], ones16[:tsz, :].unsqueeze(2))
                        nc.vector.tensor_copy(
                            va[:, :, HD + 1:HW], zcol[:tsz, 0:16].unsqueeze(2))

            def proj_v_tt(w, x, tt, pool, tag, bv_sb):
                tsz = 128 if tt < 6 else 1
                ps = pool.tile([128, 2, 512], f32, tag=tag,
                               name=f"psv{tt}")
                for ci in range(NCI):
                    for hf in (0, 1):
                        nc.tensor.matmul(
                            ps[:tsz, hf, :],
                            x[:, ci, tt * 128:tt * 128 + tsz],
                            w[ci][:, hf * 512:hf * 512 + 512],
                            start=(ci == 0), stop=(ci == NCI - 1))
                va = vaug[:tsz, tt, :].rearrange("p (h e) -> p h e", e=HW)
                nc.vector.tensor_add(
                    va[:, :, 0:HD],
                    ps[:tsz, :, :].rearrange("p r c -> p (r c)")
                        .rearrange("p (h d) -> p h d", h=H),
                    bv_sb[:tsz, :].rearrange("p (h d) -> p h d", h=H))
                nc.vector.tensor_copy(
                    va[:, :, HD:HD + 1], ones16[:tsz, :].unsqueeze(2))
                nc.vector.tensor_copy(
                    va[:, :, HD + 1:HW], zcol[:tsz, 0:16].unsqueeze(2))

            def st_chunk(j, nk, pts, psS, pt_pool):
                """Emit S^T + exp for one (pair, kv-block) chunk."""
                if nk < 6:
                    qlo, moff = R0SUB[nk]
                    ks = slice(nk * 128, (nk + 1) * 128)
                    st0 = psS.tile([128, 2, 512], f32, tag="stR0",
                                   name=f"stR0_{j}_{nk}")
                    pt0 = pt_pool.tile([128, 2, 512], bf16,
                                       tag=f"ptR0{nk}", bufs=2,
                                       name=f"ptR0_{j}_{nk}")
                    nc.tensor.matmul(
                        st0[:, 0, qlo:512], kT[0:64, j, ks],
                        qT[0:64, j, qlo:512], start=True, stop=True)
                    nc.tensor.matmul(
                        st0[:, 1, qlo:512], kT[64:128, j, ks],
                        qT[64:128, j, qlo:512], start=True, stop=True)
                    nc.scalar.activation(
                        out=pt0[:, :, qlo:512], in_=st0[:, :, qlo:512],
                        func=Exp, scale=0.125)
                    if moff is not None:
                        nc.gpsimd.tensor_mul(
                            pt0[:, 0, moff:moff + 128],
                            pt0[:, 0, moff:moff + 128], m0_sb)
                        nc.gpsimd.tensor_mul(
                            pt0[:, 1, moff:moff + 128],
                            pt0[:, 1, moff:moff + 128], m0_sb)
                    st1 = psS.tile([128, 2, 512], f32, tag="stR1",
                                   name=f"stR1_{j}_{nk}")
                    pt1 = pt_pool.tile([128, 2, 512], bf16,
                                       tag=f"ptR1{nk}", bufs=2,
                                       name=f"ptR1_{j}_{nk}")
                    nc.tensor.matmul(
                        st1[:, 0, 0:258], kT[0:64, j, ks],
                        qT[0:64, j, 512:TP], start=True, stop=True)
                    nc.tensor.matmul(
                        st1[:, 1, 0:258], kT[64:128, j, ks],
                        qT[64:128, j, 512:TP], start=True, stop=True)
                    nc.scalar.activation(
                        out=pt1[:, :, 0:258], in_=st1[:, :, 0:258],
                        func=Exp, scale=0.125)
                    pts[nk] = (pt0, pt1)
                else:
                    # kv row 768: q cols 512:770 only, col 512 masked
                    st1 = psS.tile([128, 2, 512], f32, tag="stR1",
                                   name=f"stR1_{j}_{nk}")
                    pt1 = pt_pool.tile([128, 2, 512], bf16,
                                       tag="ptR16", bufs=2,
                                       name=f"ptR1_{j}_{nk}")
                    nc.tensor.matmul(
                        st1[0:1, 0, 0:258], kT[0:64, j, 768:769],
                        qT[0:64, j, 512:TP], start=True, stop=True)
                    nc.tensor.matmul(
                        st1[0:1, 1, 0:258], kT[64:128, j, 768:769],
                        qT[64:128, j, 512:TP], start=True, stop=True)
                    nc.scalar.activation(
                        out=pt1[0:1, :, 0:258], in_=st1[0:1, :, 0:258],
                        func=Exp, scale=0.125)
                    nc.gpsimd.tensor_copy(
                        pt1[0:1, :, 0:1], zcol[0:1, 0:2].unsqueeze(2))
                    pts[nk] = (None, pt1)

            def pv_chunk(j, pts, ov0, ov1, nk):
                """One kv-block of both heads' PV accumulation."""
                pt0, pt1 = pts[nk]
                for h in (0, 1):
                    vs = slice((2 * j + h) * HW, (2 * j + h) * HW + HW)
                    if nk < 6:
                        qlo = R0SUB[nk][0]
                        nc.tensor.matmul(
                            ov0[0:HW, h, qlo:512], vaug[:, nk, vs],
                            pt0[:, h, qlo:512],
                            start=(nk == 0), stop=(nk == 5))
                        nc.tensor.matmul(
                            ov1[0:HW, h, 0:258], vaug[:, nk, vs],
                            pt1[:, h, 0:258],
                            start=(nk == 0), stop=False)
                    else:
                        nc.tensor.matmul(
                            ov1[0:HW, h, 0:258], vaug[0:1, 6, vs],
                            pt1[0:1, h, 0:258],
                            start=False, stop=True)

            def division(j, ov0, ov1, rdbc_pool, dnd, pending):
                # Softmax division, decoupled from PSUM residency:
                # 1) one DVE copy per ov chunk moves rows 0:65 (y + denom)
                #    to SBUF right after the PV stop -> ov banks free ~1.5us
                #    after the last PV matmul, so the next head pair's PV is
                #    not dammed by the broadcast round trip.
                # 2) denominator rows go SBUF->DRAM->broadcast, reciprocal.
                # 3) the normalization mults for pair j run one pair LATE
                #    (emitted in pair j+1), when rdbc(j) is long since ready.
                yraw0 = yraw_pool.tile([128, 2, 512], f32, tag="yraw0",
                                       bufs=2, name=f"yraw0_{j}")
                yraw1 = yraw_pool.tile([128, 2, 258], f32, tag="yraw1",
                                       bufs=2, name=f"yraw1_{j}")
                rdbcA = rdbc_pool.tile([64, TP], f32, tag="rdbcA",
                                       name=f"rdbcA{j}")
                rdbcB = rdbc_pool.tile([64, TP], f32, tag="rdbcB",
                                       name=f"rdbcB{j}")
                nc.vector.tensor_copy(
                    yraw0[0:HD + 1, :, :], ov0[0:HD + 1, :, :])
                nc.vector.tensor_copy(
                    yraw1[0:HD + 1, :, :], ov1[0:HD + 1, :, 0:258])
                nc.sync.dma_start(
                    out=dnd[j:j + 1, :, 0:512], in_=yraw0[HD:HD + 1, :, :])
                nc.sync.dma_start(
                    out=dnd[j:j + 1, :, 512:TP], in_=yraw1[HD:HD + 1, :, :])
                nc.gpsimd.dma_start(
                    out=rdbcA[:, 0:512],
                    in_=dnd[j:j + 1, 0, 0:512].broadcast_to((64, 512)))
                nc.gpsimd.dma_start(
                    out=rdbcB[:, 0:512],
                    in_=dnd[j:j + 1, 1, 0:512].broadcast_to((64, 512)))
                nc.gpsimd.dma_start(
                    out=rdbcA[:, 512:TP],
                    in_=dnd[j:j + 1, 0, 512:TP].broadcast_to((64, 258)))
                nc.gpsimd.dma_start(
                    out=rdbcB[:, 512:TP],
                    in_=dnd[j:j + 1, 1, 512:TP].broadcast_to((64, 258)))
                nc.vector.reciprocal_approx_fast(out=rdbcA, in_=rdbcA)
                nc.vector.reciprocal_approx_fast(out=rdbcB, in_=rdbcB)
                pending.append((j, yraw0, yraw1, rdbcA, rdbcB))

            def norm_muls(j, yraw0, yraw1, rdbcA, rdbcB):
                nc.vector.tensor_mul(
                    yT[0:64, j, 0:512], yraw0[0:HD, 0, :], rdbcA[:, 0:512])
                nc.vector.tensor_mul(
                    yT[64:128, j, 0:512], yraw0[0:HD, 1, :],
                    rdbcB[:, 0:512])
                nc.vector.tensor_mul(
                    yT[0:64, j, 512:TP], yraw1[0:HD, 0, :],
                    rdbcA[:, 512:TP])
                nc.vector.tensor_mul(
                    yT[64:128, j, 512:TP], yraw1[0:HD, 1, :],
                    rdbcB[:, 512:TP])

            def proj_out(w, psS, psO, opool):
                # psum tiles reuse the attention tags (same 2-bank shape):
                # stR0/stR1 free right after the last exps, ovR0/ovR1 after
                # the last division -> out-proj overlaps attention's tail.
                tags = ((psS, "stR0"), (psS, "stR1"), (psO, "ovR0"),
                        (psO, "ovR1"))
                pss = {}
                for co in range(NCO):
                    pool, tag = tags[co % 4]
                    pss[co] = pool.tile([128, 2, 512], f32, tag=tag,
                                        name=f"pso{co}")
                    for ci in range(NCI):
                        for r, (lo, hi) in enumerate(((0, 512), (512, TP))):
                            nc.tensor.matmul(
                                pss[co][:, r, 0:hi - lo],
                                w[ci][:, co * 128:(co + 1) * 128],
                                yT[:, ci, lo:hi],
                                start=(ci == 0), stop=(ci == NCI - 1))
                    ot = opool.tile([128, TP], bf16, tag="ot", name="ot")
                    nc.scalar.activation(
                        out=ot[:, 0:512], in_=pss[co][:, 0, :], func=Ident,
                        bias=bp_sb[:, co:co + 1], scale=1.0)
                    nc.scalar.activation(
                        out=ot[:, 512:TP], in_=pss[co][:, 1, 0:258],
                        func=Ident, bias=bp_sb[:, co:co + 1], scale=1.0)
                    nc.sync.dma_start(
                        out=out_d[co * 128:(co + 1) * 128, :], in_=ot)

            # ---- phase 1: projections ----
            with (
                tc.tile_pool(name="psA", bufs=4, space="PSUM") as psA,
                tc.tile_pool(name="xq", bufs=1) as xqp,
                tc.tile_pool(name="xkv", bufs=1) as xkp,
                tc.tile_pool(name="shpool", bufs=2) as shpool,
                tc.tile_pool(name="wqkp", bufs=1) as wqkp,
            ):
                bv_sb = qkpool.tile([128, C], f32, tag="bv")
                nc.gpsimd.dma_start(
                    out=bv_sb, in_=bv_d[0:1, :].broadcast_to((128, C)))
                xq = xqp.tile([128, NCI, TP], bf16, tag="xq")
                xkv = qkpool.tile([128, NCI, TP], bf16, tag="xkv")
                # HWDGE rings are FIFO per engine: SP ring carries xq+wq
                # (per-ci interleaved) then wv; ACT ring carries xkv+wk
                # (per-ci interleaved) then wp. Need-ordering keeps the
                # critical QK-proj inputs at full HBM bandwidth instead of
                # round-robin-sharing it with the late-needed wv/wp.
                # SP ring: xq+wq per-ci interleaved (+ the rotary shuffles
                # issued on dependency). ACT ring: xkv+wk per-ci, then wv,
                # then wp — ring FIFO delays wv/wp past the critical K-proj
                # inputs without blocking any compute queue.
                wq, wk = [], []
                for ci in range(NCI):
                    nc.sync.dma_start(
                        out=xq[:, ci, :],
                        in_=xqT_d[ci * 128:(ci + 1) * 128, :])
                    wt = wqkp.tile([128, C], bf16, tag=f"wq{ci}",
                                   name=f"wq{ci}")
                    nc.sync.dma_start(
                        out=wt, in_=wq_d[ci * 128:(ci + 1) * 128, :])
                    wq.append(wt)
                    nc.scalar.dma_start(
                        out=xkv[:, ci, :],
                        in_=xkvT_d[ci * 128:(ci + 1) * 128, :])
                    wt = wqkp.tile([128, C], bf16, tag=f"wk{ci}",
                                   name=f"wk{ci}")
                    nc.scalar.dma_start(
                        out=wt, in_=wk_d[ci * 128:(ci + 1) * 128, :])
                    wk.append(wt)
                proj_qk(wq, xq, bq_sb, qT, psA, shpool)
                # wv/wp enqueue on the HWDGE rings BEHIND the critical
                # xq/wq/xkv/wk transfers (ring FIFO = delayed start at full
                # bandwidth), split across both rings, wv strictly first.
                wv, wp = [], []
                for ci in range(NCI):
                    eng = nc.sync if ci % 2 == 0 else nc.scalar
                    wt = wpool.tile([128, C], bf16, tag=f"wv{ci}",
                                    name=f"wv{ci}")
                    eng.dma_start(
                        out=wt, in_=wv_d[ci * 128:(ci + 1) * 128, :])
                    wv.append(wt)
                for ci in range(NCI):
                    eng = nc.sync if ci % 2 == 0 else nc.scalar
                    wt = wpool.tile([128, C], bf16, tag=f"wp{ci}",
                                    name=f"wp{ci}")
                    eng.dma_start(
                        out=wt, in_=wp_d[ci * 128:(ci + 1) * 128, :])
                    wp.append(wt)
                proj_qk(wk, xkv, bk_sb, kT, psA, shpool)
                wv_tiles = wv
                wp_tiles = wp
                bv_keep = bv_sb

            # ---- phases 2+3 share yT ----
            with tc.tile_pool(name="ypool", bufs=1) as ypool:
                yT = ypool.tile([128, NCI, TP], bf16, tag="yT")
                # ---- phase 2: attention ----
                with (
                    tc.tile_pool(name="psS", bufs=1, space="PSUM") as psS,
                    tc.tile_pool(name="psO", bufs=1, space="PSUM") as psO,
                    tc.tile_pool(name="ptp", bufs=1) as pt_pool,
                    tc.tile_pool(name="rdbcp", bufs=2) as rdbc_pool,
                    tc.tile_pool(name="yrawp", bufs=2) as yraw_pool,
                    tc.tile_pool(name="rddp", bufs=1, space="DRAM") as dram_pool,
                ):
                    dnd = dram_pool.tile([NCI, 2, TP], f32, tag="dnd")
                    pending = []
                    ptsd = {j: {} for j in range(NCI)}
                    # Prime: V-projection chunks (PE filler, no dependence on
                    # the K rotary) run first so the PE rides through the
                    # K-rotary pipeline drain; pairs 0-1's S^T+exp interleave
                    # behind them. VP chunks reuse the st-ring psum tags.
                    for tt in (0, 1, 2):
                        proj_v_tt(wv_tiles, xkv, tt, psS,
                                  "stR0" if tt % 2 == 0 else "stR1", bv_keep)
                    for nk in (0, 1, 2, 3):
                        st_chunk(0, nk, ptsd[0], psS, pt_pool)
                    proj_v_tt(wv_tiles, xkv, 3, psS, "stR1", bv_keep)
                    for nk in (4, 5, 6):
                        st_chunk(0, nk, ptsd[0], psS, pt_pool)
                    proj_v_tt(wv_tiles, xkv, 4, psS, "stR0", bv_keep)
                    for nk in (0, 1, 2, 3):
                        st_chunk(1, nk, ptsd[1], psS, pt_pool)
                    proj_v_tt(wv_tiles, xkv, 5, psS, "stR1", bv_keep)
                    for nk in (4, 5, 6):
                        st_chunk(1, nk, ptsd[1], psS, pt_pool)
                    proj_v_tt(wv_tiles, xkv, 6, psS, "stR0", bv_keep)
                    # Steady state: PV of pair j chunk-interleaved with
                    # S^T+exp of pair j+2, so the PE never idles while ACT
                    # (the attention-phase bottleneck) drains exp chunks.
                    for j in range(NCI):
                        ov0 = psO.tile([128, 2, 512], f32, tag="ovR0",
                                       name=f"ov0_{j}")
                        ov1 = psO.tile([128, 2, 512], f32, tag="ovR1",
                                       name=f"ov1_{j}")
                        for nk in range(7):
                            pv_chunk(j, ptsd[j], ov0, ov1, nk)
                            if j + 2 < NCI:
                                st_chunk(j + 2, nk, ptsd[j + 2], psS, pt_pool)
                        division(j, ov0, ov1, rdbc_pool, dnd, pending)
                        if len(pending) > 1:
                            norm_muls(*pending.pop(0))
                    for args in pending:
                        norm_muls(*args)

                    # ---- phase 3: output projection (attention psum tags
                    # reused so it overlaps the tail of attention) ----
                    with tc.tile_pool(name="opool", bufs=3) as opool:
                        proj_out(wp_tiles, psS, psO, opool)

    nc.compile()
    return nc


def _host_prep(x_q, x_kv, rotary_pos_emb, Wq, bq, Wk, bk, Wv, bv, Wp, bp):
    f = np.float32
    x_q = np.asarray(x_q, f)
    x_kv = np.asarray(x_kv, f)
    freqs = np.asarray(rotary_pos_emb, f)

    # Even/odd pair-split permutation of the first 32 dims of each head, so
    # rotate_half becomes a 16-partition block swap on chip.
    perm = np.arange(C)
    for h in range(H):
        b0 = h * HD
        blk = np.empty(HD, np.int64)
        blk[0:16] = b0 + np.arange(0, 32, 2)
        blk[16:32] = b0 + np.arange(1, 32, 2)
        blk[32:64] = b0 + np.arange(32, 64)
        perm[b0:b0 + HD] = blk

    def wT(W, p=None):
        W = np.asarray(W, f)
        if p is not None:
            W = W[p, :]
        return np.ascontiguousarray(W.T).astype(np.float32)

    cosE = np.cos(freqs[:, 0::2]).T
    cosO = np.cos(freqs[:, 1::2]).T
    sinE = -np.sin(freqs[:, 0::2]).T
    sinO = np.sin(freqs[:, 1::2]).T
    cosP = np.ones((128, TP), f)
    sinP = np.zeros((128, TP), f)
    for s in (0, 64):
        cosP[s:s + 16, :T] = cosE
        cosP[s + 16:s + 32, :T] = cosO
        sinP[s:s + 16, :T] = sinE
        sinP[s + 16:s + 32, :T] = sinO

    p_idx = np.arange(128)[:, None]
    f_idx = np.arange(128)[None, :]
    m0 = (p_idx < f_idx).astype(f)

    import ml_dtypes
    bf = ml_dtypes.bfloat16

    bqp = np.asarray(bq, f)[perm]
    bkp = np.asarray(bk, f)[perm]
    shared = {
        "wqT": wT(Wq, perm).astype(bf),
        "wkT": wT(Wk, perm).astype(bf),
        "wvT": wT(Wv).astype(bf),
        "wpT": wT(Wp).astype(bf),
        "bq2": np.ascontiguousarray(bqp.reshape(NCO, 128).T).astype(f),
        "bk2": np.ascontiguousarray(bkp.reshape(NCO, 128).T).astype(f),
        "bp2": np.ascontiguousarray(
            np.asarray(bp, f).reshape(NCO, 128).T).astype(f),
        "bv1": np.asarray(bv, f).reshape(1, C).copy(),
        "cosP": np.ascontiguousarray(cosP).astype(bf),
        "sinP": np.ascontiguousarray(sinP).astype(bf),
        "m0": np.ascontiguousarray(m0).astype(bf),
    }

    def padT(xt):
        out = np.zeros((C, TP), np.float32)
        out[:, :T] = xt
        return out.astype(bf)

    in_maps = []
    for b in range(B):
        m = dict(shared)
        m["xqT"] = padT(x_q[b].T)
        m["xkvT"] = padT(x_kv[b].T)
        in_maps.append(m)
    return in_maps


def kernel(x_q, x_kv, rotary_pos_emb, Wq, bq, Wk, bk, Wv, bv, Wp, bp):
    from concourse.bass_utils import run_bass_kernel_spmd

    if "nc" not in _CACHE:
        _CACHE["nc"] = _build_program()
    nc = _CACHE["nc"]

    in_maps = _host_prep(x_q, x_kv, rotary_pos_emb,
                         Wq, bq, Wk, bk, Wv, bv, Wp, bp)
    trace = os.environ.get("BTK_TRACE", "0") == "1"
    res = run_bass_kernel_spmd(
        nc, in_maps, core_ids=list(range(B)), trace=trace)
    _CACHE["last_result"] = res
    out = np.stack(
        [np.asarray(r["outT"][:, :T], np.float32).T for r in res.results],
        axis=0)
    return out

